# revision 15
# baseline (speedup 1.0000x reference)
"""Memory-attention Bass kernel for Trainium2, SPMD over 8 NeuronCores.

Sharding: pure data parallel — core i computes batch element i (B == 8).
Compute dtype: float32r on the TensorEngine (1 cycle/row vs 4 for fp32),
fp32 accumulation in PSUM, exp on ScalarE with the 1/sqrt(D) scale folded
into the activation's affine stage.

Layout strategy ("T-layout" = feature-on-partition):
  x1T/x2T  : PE-transposed inputs               [din, tok]
  qT, kT   : projections produced in T-layout   [dout, tok]   (lhsT = W)
  v        : produced in natural layout         [ktok, dout]  (lhsT = x2T)
  S^T      : [ktok, qtok] per head, two heads per 128-row tile via K=64
             row tiling of the PE array
  P^T      : exp(S^T * scale) via one ACTIVATE per (pair, qchunk, ktile)
  O^T      : PV with M=64 col tiling -> both heads stacked in one PSUM tile
  sums     : ones-column matmuls into rows {0, 64} of a PSUM tile
  final    : natural layout (lhsT = attnT, rhs = Wp), + bias, DMA out
"""

import sys

import numpy as np

sys.path.insert(0, "/opt/trn_rl_repo")

import concourse.bacc as bacc
import concourse.mybir as mybir
import concourse.tile as tile
from concourse.bass_utils import run_bass_kernel_spmd

F32 = mybir.dt.float32
F32R = mybir.dt.float32r
EXP = mybir.ActivationFunctionType.Exp
MULT = mybir.AluOpType.mult
ADD = mybir.AluOpType.add

DIM = 1024
H = 16
D = 64
N1 = 1024
N2 = 1024
SLOTS = 64
NKV = N2 + SLOTS  # 1088
P = 128
NT = DIM // P     # 8 feature/token tiles
NPAIR = H // 2    # 8 head pairs (2 heads of 64 dims = one 128-partition tile)
KTILES = NKV // P + 1  # 8 full kv tiles + 1 tile of 64 memory slots
SCALE = float(D) ** -0.5
NCORES = 8


def build_nc():
    nc = bacc.Bacc(None)

    x1 = nc.declare_dram_parameter("x1", [N1, DIM], F32, isOutput=False)
    x2 = nc.declare_dram_parameter("x2", [N2, DIM], F32, isOutput=False)
    mk = nc.declare_dram_parameter("memory_k", [SLOTS, DIM], F32, isOutput=False)
    mv = nc.declare_dram_parameter("memory_v", [SLOTS, DIM], F32, isOutput=False)
    Wq = nc.declare_dram_parameter("Wq", [DIM, DIM], F32, isOutput=False)
    bq = nc.declare_dram_parameter("bq", [DIM], F32, isOutput=False)
    Wk = nc.declare_dram_parameter("Wk", [DIM, DIM], F32, isOutput=False)
    bk = nc.declare_dram_parameter("bk", [DIM], F32, isOutput=False)
    Wv = nc.declare_dram_parameter("Wv", [DIM, DIM], F32, isOutput=False)
    bv = nc.declare_dram_parameter("bv", [DIM], F32, isOutput=False)
    Wp = nc.declare_dram_parameter("Wp", [DIM, DIM], F32, isOutput=False)
    bp = nc.declare_dram_parameter("bp", [DIM], F32, isOutput=False)
    eye = nc.declare_dram_parameter("eye128", [P, P], F32, isOutput=False)
    out = nc.declare_dram_parameter("out", [N1, DIM], F32, isOutput=True)

    with tile.TileContext(nc) as tc:
        with (
            tc.tile_pool(name="const", bufs=1) as const,
            tc.tile_pool(name="stage", bufs=2) as stage,
            tc.tile_pool(name="kTp", bufs=1) as kT_pool,
            tc.tile_pool(name="vp", bufs=1) as v_pool,
            tc.tile_pool(name="qTp", bufs=2) as qT_pool,
            tc.tile_pool(name="PTp", bufs=2) as pt_pool,
            tc.tile_pool(name="rbp", bufs=2) as rb_pool,
            tc.tile_pool(name="ps_big", bufs=3, space="PSUM") as ps_big,
            tc.tile_pool(name="ps_one", bufs=2, space="PSUM") as ps_one,
        ):
            # ---- constants / biases ----
            eye_sb = const.tile([P, P], F32, tag="eye")
            nc.sync.dma_start(eye_sb[:], eye[:])
            ones_f = const.tile([P, H], F32, tag="ones_f")
            nc.vector.memset(ones_f[:], 1.0)

            bq_sb = const.tile([P, NT], F32, tag="bq")
            nc.sync.dma_start(bq_sb[:], bq.rearrange("(t p) -> p t", p=P))
            bk_sb = const.tile([P, NT], F32, tag="bk")
            nc.sync.dma_start(bk_sb[:], bk.rearrange("(t p) -> p t", p=P))
            BCAST0 = [0] * 32

            def double_up(tl, base, rows, width):
                """Broadcast row `base` (already written) to rows [base, base+rows).

                base and rows must be multiples of 32 (except rows==... uses
                stream_shuffle within the first quadrant, then aligned copies).
                """
                nc.vector.stream_shuffle(
                    tl[base : base + 32, 0:width], tl[base : base + 32, 0:width], BCAST0
                )
                n = 32
                while n < rows:
                    c = min(n, rows - n)
                    nc.vector.tensor_copy(
                        tl[base + n : base + n + c, 0:width], tl[base : base + c, 0:width]
                    )
                    n += c

            bv_b = const.tile([P, DIM], F32, tag="bv_b")
            nc.sync.dma_start(bv_b[0:1, :], bv.rearrange("(a f) -> a f", a=1))
            double_up(bv_b, 0, P, DIM)
            bp_b = const.tile([P, DIM], F32, tag="bp_b")
            nc.sync.dma_start(bp_b[0:1, :], bp.rearrange("(a f) -> a f", a=1))
            double_up(bp_b, 0, P, DIM)

            # ---- persistent activation tensors ----
            # v_sb: per head 65 columns = 64 value dims + a ones column, so the
            # PV matmul (M=65) yields softmax denominators in output row 64.
            VW = H * (D + 1)  # 1040
            kT = [kT_pool.tile([P, NKV], F32R, tag=f"kT{p}", name=f"kT{p}") for p in range(NT)]
            v_r = [v_pool.tile([P, VW], F32R, tag=f"v{t}", name=f"v{t}") for t in range(NT)]
            mv_r = v_pool.tile([SLOTS, VW], F32R, tag="mv", name="mv")

            def v_strided(tile_, rows):
                return tile_[0:rows, :].rearrange("p (h c) -> p h c", c=D + 1)

            def transpose_into(dstT, src_stage, t):
                """dstT[p][:, t*128:(t+1)*128] = src tile t transposed (8 PE ops)."""
                for p in range(NT):
                    tp = ps_one.tile([P, P], F32, tag="pv")
                    nc.tensor.transpose(
                        tp[:], src_stage[:, p * P : (p + 1) * P], eye_sb[:]
                    )
                    nc.any.tensor_copy(dstT[p][:, t * P : (t + 1) * P], tp[:])

            with tc.tile_pool(name="x2Tp", bufs=1) as x2T_pool:
                x2T = [x2T_pool.tile([P, N2], F32R, tag=f"x2T{j}", name=f"x2T{j}") for j in range(NT)]
                wv_r = [
                    x2T_pool.tile([P, DIM], F32R, tag=f"wv{j}", name=f"wv{j}") for j in range(NT)
                ]

                # x2 -> x2T (PE transpose)
                for t in range(NT):
                    xs = stage.tile([P, DIM], F32, tag="xstage")
                    nc.sync.dma_start(xs[:], x2[t * P : (t + 1) * P, :])
                    transpose_into(x2T, xs, t)

                # Wv row tiles -> f32r
                for j in range(NT):
                    wvf = stage.tile([P, DIM], F32, tag="xstage")
                    nc.sync.dma_start(wvf[:], Wv[j * P : (j + 1) * P, :])
                    nc.any.tensor_copy(wv_r[j][:], wvf[:])

                # memory_k -> kT[:, 1024:1088]
                mks = stage.tile([SLOTS, DIM], F32, tag="mstage")
                nc.sync.dma_start(mks[:], mk[:])
                for p in range(NT):
                    tp = ps_one.tile([P, SLOTS], F32, tag="pv")
                    nc.tensor.transpose(
                        tp[:, 0:SLOTS],
                        mks[0:SLOTS, p * P : (p + 1) * P],
                        eye_sb[0:SLOTS, 0:SLOTS],
                    )
                    nc.any.tensor_copy(kT[p][:, N2:NKV], tp[:, 0:SLOTS])

                # memory_v -> f32r natural (strided per-head layout + ones col)
                mvs = stage.tile([SLOTS, DIM], F32, tag="mstage")
                nc.sync.dma_start(mvs[:], mv[:])
                nc.any.tensor_copy(
                    v_strided(mv_r, SLOTS)[:, :, 0:D],
                    mvs[0:SLOTS, :].rearrange("p (h c) -> p h c", c=D),
                )
                nc.vector.tensor_copy(
                    v_strided(mv_r, SLOTS)[:, :, D : D + 1], ones_f[0:SLOTS, :]
                )

                # k-projection: kT[p] = (Wk[:, pcols].T @ x2T) + bk  (T-layout)
                for p in range(NT):
                    kps = ps_big.tile([P, N2], F32, tag="big")
                    for j in range(NT):
                        wkf = stage.tile([P, P], F32, tag="wstage")
                        nc.sync.dma_start(
                            wkf[:], Wk[j * P : (j + 1) * P, p * P : (p + 1) * P]
                        )
                        wkr = stage.tile([P, P], F32R, tag="wkr")
                        nc.any.tensor_copy(wkr[:], wkf[:])
                        for c in range(2):
                            nc.tensor.matmul(
                                kps[:, c * 512 : (c + 1) * 512],
                                wkr[:],
                                x2T[j][:, c * 512 : (c + 1) * 512],
                                start=(j == 0),
                                stop=(j == NT - 1),
                            )
                    nc.vector.tensor_scalar_add(
                        kT[p][:, 0:N2], kps[:], bk_sb[:, p : p + 1]
                    )

                # v-projection (natural layout): v[t] = x2[trows] @ Wv + bv
                for t in range(NT):
                    vps = ps_big.tile([P, DIM], F32, tag="big")
                    for j in range(NT):
                        for c in range(2):
                            nc.tensor.matmul(
                                vps[:, c * 512 : (c + 1) * 512],
                                x2T[j][:, t * P : (t + 1) * P],
                                wv_r[j][:, c * 512 : (c + 1) * 512],
                                start=(j == 0),
                                stop=(j == NT - 1),
                            )
                    nc.vector.tensor_tensor(
                        v_strided(v_r[t], P)[:, :, 0:D],
                        vps[:].rearrange("p (h c) -> p h c", c=D),
                        bv_b[:].rearrange("p (h c) -> p h c", c=D),
                        op=ADD,
                    )
                    nc.vector.tensor_copy(
                        v_strided(v_r[t], P)[:, :, D : D + 1], ones_f[:]
                    )

            # ---- attention pairs ----
            attnT_cm = tc.tile_pool(name="attnTp", bufs=1)
            attnT_pool = attnT_cm.__enter__()
            attnT = [
                attnT_pool.tile([P, N1], F32R, tag=f"attnT{p}", name=f"attnT{p}") for p in range(NT)
            ]
            with (
                tc.tile_pool(name="x1Tp", bufs=1) as x1T_pool,
                tc.tile_pool(name="wqp", bufs=3) as wq_pool,
            ):
                x1T = [x1T_pool.tile([P, N1], F32R, tag=f"x1T{j}", name=f"x1T{j}") for j in range(NT)]

                for t in range(NT):
                    xs = stage.tile([P, DIM], F32, tag="xstage")
                    nc.sync.dma_start(xs[:], x1[t * P : (t + 1) * P, :])
                    transpose_into(x1T, xs, t)

                for pr in range(NPAIR):
                    # q-projection for this pair (T-layout, streamed Wq cols)
                    qps = ps_big.tile([P, N1], F32, tag="big")
                    for j in range(NT):
                        wqf = stage.tile([P, P], F32, tag="wstage")
                        nc.sync.dma_start(
                            wqf[:], Wq[j * P : (j + 1) * P, pr * P : (pr + 1) * P]
                        )
                        wqr = wq_pool.tile([P, P], F32R, tag="wqr")
                        nc.any.tensor_copy(wqr[:], wqf[:])
                        for c in range(2):
                            nc.tensor.matmul(
                                qps[:, c * 512 : (c + 1) * 512],
                                wqr[:],
                                x1T[j][:, c * 512 : (c + 1) * 512],
                                start=(j == 0),
                                stop=(j == NT - 1),
                            )
                    qTt = qT_pool.tile([P, N1], F32R, tag="qT")
                    nc.vector.tensor_scalar_add(qTt[:], qps[:], bq_sb[:, pr : pr + 1])

                    colA = 2 * pr * (D + 1)
                    colB = (2 * pr + 1) * (D + 1)
                    for qc in range(2):
                        qsl = slice(qc * 512, (qc + 1) * 512)
                        pvA = ps_one.tile([D + 1, 512], F32, tag="pv")
                        pvB = ps_one.tile([D + 1, 512], F32, tag="pv")
                        for kt in range(KTILES):
                            m = P if kt < KTILES - 1 else SLOTS
                            koff = kt * P
                            sps = ps_big.tile([P, 1024], F32, tag="big")
                            # S^T tiles for both heads (K=64 row tiling)
                            nc.tensor.matmul(
                                sps[0:m, 0:512],
                                kT[pr][0:D, koff : koff + m],
                                qTt[0:D, qsl],
                                start=True,
                                stop=True,
                                tile_position=(0, 0),
                            )
                            nc.tensor.matmul(
                                sps[0:m, 512:1024],
                                kT[pr][D:P, koff : koff + m],
                                qTt[D:P, qsl],
                                start=True,
                                stop=True,
                                tile_position=(64, 0),
                            )
                            # P^T = exp(S^T * scale)
                            ptt = pt_pool.tile([P, 1024], F32R, tag="PT")
                            nc.scalar.activation(
                                ptt[0:m, :], sps[0:m, :], EXP, scale=SCALE
                            )
                            vsrc = v_r[kt] if kt < KTILES - 1 else mv_r
                            # O^T per head, M=65: row 64 = softmax denominator
                            nc.tensor.matmul(
                                pvA[:, :],
                                vsrc[0:m, colA : colA + D + 1],
                                ptt[0:m, 0:512],
                                start=(kt == 0),
                                stop=(kt == KTILES - 1),
                            )
                            nc.tensor.matmul(
                                pvB[:, :],
                                vsrc[0:m, colB : colB + D + 1],
                                ptt[0:m, 512:1024],
                                start=(kt == 0),
                                stop=(kt == KTILES - 1),
                            )
                        # normalize: attnT = O^T * (1/sums)
                        rb = rb_pool.tile([P, 512], F32, tag="rb")
                        nc.vector.reciprocal(rb[0:1, :], pvA[D : D + 1, :])
                        nc.vector.reciprocal(rb[D : D + 1, :], pvB[D : D + 1, :])
                        double_up(rb, 0, D, 512)
                        double_up(rb, D, D, 512)
                        nc.vector.tensor_tensor(
                            attnT[pr][0:D, qsl], pvA[0:D, :], rb[0:D, :], op=MULT
                        )
                        nc.vector.tensor_tensor(
                            attnT[pr][D:P, qsl], pvB[0:D, :], rb[D:P, :], op=MULT
                        )

            # ---- final projection (natural layout) ----
            with (
                tc.tile_pool(name="wpp", bufs=1) as wp_pool,
                tc.tile_pool(name="outp", bufs=2) as out_pool,
            ):
                wp_r = [wp_pool.tile([P, DIM], F32R, tag=f"wp{j}", name=f"wp{j}") for j in range(NT)]
                for j in range(NT):
                    wpf = stage.tile([P, DIM], F32, tag="xstage")
                    nc.sync.dma_start(wpf[:], Wp[j * P : (j + 1) * P, :])
                    nc.any.tensor_copy(wp_r[j][:], wpf[:])
                for t in range(NT):
                    ops = ps_big.tile([P, DIM], F32, tag="big")
                    for j in range(NT):
                        for c in range(2):
                            nc.tensor.matmul(
                                ops[:, c * 512 : (c + 1) * 512],
                                attnT[j][:, t * P : (t + 1) * P],
                                wp_r[j][:, c * 512 : (c + 1) * 512],
                                start=(j == 0),
                                stop=(j == NT - 1),
                            )
                    osb = out_pool.tile([P, DIM], F32, tag="osb")
                    nc.vector.tensor_tensor(osb[:], ops[:], bp_b[:], op=ADD)
                    nc.sync.dma_start(out[t * P : (t + 1) * P, :], osb[:])
            attnT_cm.__exit__(None, None, None)

    nc.finalize()
    return nc


def make_in_maps(inputs):
    """Per-core input dicts: core i gets batch element i, weights replicated."""
    a = {k: np.ascontiguousarray(np.asarray(v, dtype=np.float32)) for k, v in inputs.items()}
    eye = np.eye(P, dtype=np.float32)
    maps = []
    for i in range(NCORES):
        maps.append(
            {
                "x1": a["x1"][i],
                "x2": a["x2"][i],
                "memory_k": a["memory_k"][i],
                "memory_v": a["memory_v"][i],
                "Wq": a["Wq"],
                "bq": a["bq"],
                "Wk": a["Wk"],
                "bk": a["bk"],
                "Wv": a["Wv"],
                "bv": a["bv"],
                "Wp": a["Wp"],
                "bp": a["bp"],
                "eye128": eye,
            }
        )
    return maps


_NC_CACHE = [None]


def kernel(**inputs) -> np.ndarray:
    if _NC_CACHE[0] is None:
        _NC_CACHE[0] = build_nc()
    nc = _NC_CACHE[0]
    in_maps = make_in_maps(inputs)
    res = run_bass_kernel_spmd(nc, in_maps, core_ids=list(range(NCORES)))
    return np.stack([res.results[i]["out"] for i in range(NCORES)], axis=0)


if __name__ == "__main__":
    rng = np.random.default_rng(0)
    ins = {
        "x1": rng.standard_normal((8, N1, DIM), dtype=np.float32),
        "x2": rng.standard_normal((8, N2, DIM), dtype=np.float32),
        "memory_k": rng.standard_normal((8, SLOTS, DIM), dtype=np.float32),
        "memory_v": rng.standard_normal((8, SLOTS, DIM), dtype=np.float32),
        "Wq": rng.standard_normal((DIM, DIM), dtype=np.float32) / 32.0,
        "bq": np.zeros(DIM, np.float32),
        "Wk": rng.standard_normal((DIM, DIM), dtype=np.float32) / 32.0,
        "bk": np.zeros(DIM, np.float32),
        "Wv": rng.standard_normal((DIM, DIM), dtype=np.float32) / 32.0,
        "bv": np.zeros(DIM, np.float32),
        "Wp": rng.standard_normal((DIM, DIM), dtype=np.float32) / 32.0,
        "bp": np.zeros(DIM, np.float32),
    }
    o = kernel(**ins)
    print("out", o.shape, o.dtype, float(np.abs(o).mean()))
